# revision 22
# baseline (speedup 1.0000x reference)
"""Trainium2 Bass kernel for nn_Block_30313879175568 (dense transformer block).

Sharding: (batch, head-group). Core c handles batch b = c//2 and head-group
g = c%2 (heads 8g..8g+7). QKV/attention computed locally for the whole batch
sequence; the only collective is a small pairwise AllToAll of attention
outputs o^T (+rowsum rows). Each core only needs its partner's half of the
q columns, so A2A both halves the wire bytes vs an AllGather and performs
the data-dependent column select in routing (no mask inputs needed).
Softmax normalization is deferred past the collective: the stage carries
unnormalized o and per-(head,q) rowsums; division happens in the select
phase via a fast reciprocal on [4,1024] plus gpsimd partition broadcasts.
FFN/out-proj row-parallel over the core's own 1024 rows.

Self-contained: imports only installed packages (concourse et al.) + numpy.
"""

from contextlib import ExitStack

import numpy as np
import ml_dtypes

import concourse.bass as bass  # noqa: F401
import concourse.mybir as mybir
import concourse.tile as tile
from concourse import bacc
from concourse.bass_utils import run_bass_kernel_spmd

BF16 = mybir.dt.bfloat16
F32 = mybir.dt.float32
AF = mybir.ActivationFunctionType
ALU = mybir.AluOpType

B, L, DIM, H, HID = 4, 2048, 1024, 16, 4096
HEAD_DIM = 64
NCORES = 8
R = B * L              # 8192 global rows
RC = R // NCORES       # 1024 rows per core
RMS_EPS = 1e-6
LN_EPS = 1e-5
VCOL = 2 * (HEAD_DIM + 1)   # 130: V cols per ktile block incl ones cols

_PROGRAM_CACHE = {}
_LAST_IN_MAPS = None


# ----------------------------------------------------------------------------
# host-side helpers
# ----------------------------------------------------------------------------

def _bf16(a):
    return np.asarray(a, dtype=np.float32).astype(ml_dtypes.bfloat16)


def _rope_tables():
    half = HEAD_DIM // 2
    inv_freq = 10000.0 ** (-np.arange(0, half, dtype=np.float32) * 2.0 / HEAD_DIM)
    pos = np.arange(L, dtype=np.float32)
    theta = pos[:, None] * inv_freq[None, :]          # [L, 32]
    cos = np.cos(theta).T.astype(np.float32)          # [32, L]
    sin = np.sin(theta).T.astype(np.float32)
    return (_bf16(np.tile(cos, (4, 1))), _bf16(np.tile(sin, (4, 1))))  # [128, L]


def _classify_mask(mask):
    """Split mask^T [k, q] into (16 ktile x 4 qblock) blocks of [128, 512].

    Returns (actions, pmask_np): actions[(kt, qb)] is 'skip' | 'noop' |
    ('mul', idx); pmask_np is [NU, 128, 1024] bf16 of exp(mask^T block),
    tiled twice along columns (both heads share the block).
    """
    maskT = np.asarray(mask, dtype=np.float32).T
    actions = {}
    uniq = {}
    tiles = []
    for qb in range(4):
        for kt in range(16):
            blk = maskT[128 * kt:128 * (kt + 1), 512 * qb:512 * (qb + 1)]
            if np.all(blk <= -30.0):
                actions[(kt, qb)] = "skip"
            elif np.all(blk == 0.0):
                actions[(kt, qb)] = "noop"
            else:
                pm = _bf16(np.tile(np.exp(blk.astype(np.float64)), (1, 2)))
                key = pm.tobytes()
                if key not in uniq:
                    uniq[key] = len(tiles)
                    tiles.append(pm)
                actions[(kt, qb)] = ("mul", uniq[key])
    if not tiles:
        tiles = [np.zeros((128, 1024), dtype=ml_dtypes.bfloat16)]
    pmask_np = np.stack(tiles, axis=0)
    return actions, pmask_np


def _numpy_fallback(x, mask, attn_scale, wqkv_w, wqkv_b, out_w, out_b,
                    ffn_scale, lin1_w, lin1_b, ln_g, ln_b, lin2_w, lin2_b):
    """Correct (slow) host fallback for configurations the device program
    doesn't support (nonzero biases / fully-masked rows)."""
    from scipy.special import erf

    def rms(t, scale):
        return t / np.sqrt(np.mean(t * t, axis=-1, keepdims=True) + RMS_EPS) * scale

    x = np.asarray(x, np.float64)
    xn = rms(x, attn_scale)
    qkv = xn @ np.asarray(wqkv_w, np.float64) + wqkv_b
    q, k, v = np.split(qkv, 3, axis=-1)
    th = lambda t: t.reshape(B, L, H, HEAD_DIM).transpose(0, 2, 1, 3)
    q, k, v = th(q), th(k), th(v)

    half = HEAD_DIM // 2
    inv_freq = 10000.0 ** (-np.arange(0, half) * 2.0 / HEAD_DIM)
    theta = np.arange(L)[:, None] * inv_freq[None, :]
    cos, sin = np.cos(theta), np.sin(theta)

    def rope(t):
        x1, x2 = t[..., :half], t[..., half:]
        return np.concatenate([x1 * cos - x2 * sin, x1 * sin + x2 * cos], axis=-1)

    q, k = rope(q), rope(k)
    s = np.einsum("bhqd,bhkd->bhqk", q / np.sqrt(HEAD_DIM), k) + np.asarray(mask, np.float64)
    s = s - s.max(axis=-1, keepdims=True)
    p = np.exp(s)
    p /= p.sum(axis=-1, keepdims=True)
    o = np.einsum("bhqk,bhkd->bhqd", p, v)
    o = o.transpose(0, 2, 1, 3).reshape(B, L, DIM)
    h = x + o @ np.asarray(out_w, np.float64) + out_b
    f = rms(h, ffn_scale)
    f = f @ np.asarray(lin1_w, np.float64) + lin1_b
    f = 0.5 * f * (1.0 + erf(f / np.sqrt(2.0)))
    mu = f.mean(axis=-1, keepdims=True)
    var = f.var(axis=-1, keepdims=True)
    f = (f - mu) / np.sqrt(var + LN_EPS) * ln_g + ln_b
    out = h + f @ np.asarray(lin2_w, np.float64) + lin2_b
    return out.astype(np.float32)


# ----------------------------------------------------------------------------
# device program
# ----------------------------------------------------------------------------

def _rms_rstd(nc, scratch, stats, t, eps):
    """1/sqrt(mean(t^2, free) + eps) for a [128, D] f32 tile, via ACT."""
    D = t.shape[1]
    sq = scratch.tile([128, D], BF16, tag="sq", name="sq")
    ssq = stats.tile([128, 1], F32, tag="ssq", name="ssq")
    nc.scalar.activation(sq[:], t[:], AF.Square, accum_out=ssq[:])
    std = stats.tile([128, 1], F32, tag="rmssd", name="rmssd")
    nc.scalar.activation(std[:], ssq[:], AF.Sqrt, bias=eps, scale=1.0 / D)
    rstd = stats.tile([128, 1], F32, tag="rmsrs", name="rmsrs")
    nc.vector.reciprocal(rstd[:], std[:])
    return rstd


def _register_const(nc, value, dtype=F32):
    t = nc.alloc_sbuf_tensor(f"const-{dtype.name}-{value}", [128, 1], dtype)
    nc.gpsimd.memset(t.ap(), value)
    nc.const_aps.aps[(dtype, value)] = t.ap()


def _build_program(actions, n_pmask):
    nc = bacc.Bacc("TRN2", target_bir_lowering=False, debug=False,
                   num_devices=NCORES)
    _register_const(nc, RMS_EPS)
    _register_const(nc, LN_EPS)
    nc.all_engine_barrier()

    xb_in = nc.dram_tensor("x_batch", [L, DIM], F32, kind="ExternalInput")
    x_in = nc.dram_tensor("x_own", [RC, DIM], F32, kind="ExternalInput")
    wqkv_in = nc.dram_tensor("wqkv_sl", [DIM, 4 * 384], BF16, kind="ExternalInput")
    outw_in = nc.dram_tensor("out_w", [DIM, DIM], BF16, kind="ExternalInput")
    l1w_in = nc.dram_tensor("lin1_w", [DIM, HID], BF16, kind="ExternalInput")
    l2w_in = nc.dram_tensor("lin2_w", [HID, DIM], BF16, kind="ExternalInput")
    cos_in = nc.dram_tensor("cosT", [128, L], BF16, kind="ExternalInput")
    sin_in = nc.dram_tensor("sinT", [128, L], BF16, kind="ExternalInput")
    pm_in = nc.dram_tensor("pmask", [n_pmask, 128, 1024], BF16, kind="ExternalInput")
    eye_in = nc.dram_tensor("eye", [128, 128], BF16, kind="ExternalInput")
    msel_in = nc.dram_tensor("msel", [128, 2], F32, kind="ExternalInput")
    y_out = nc.dram_tensor("y_own", [RC, DIM], F32, kind="ExternalOutput")

    with tile.TileContext(nc) as tc:
        _emit(nc, tc, xb_in, x_in, wqkv_in, outw_in, l1w_in, l2w_in,
              cos_in, sin_in, pm_in, eye_in, msel_in, y_out, actions, n_pmask)

    nc.compile()
    return nc


def _emit(nc, tc, xb_in, x_in, wqkv_in, outw_in, l1w_in, l2w_in,
          cos_in, sin_in, pm_in, eye_in, msel_in, y_out, actions, n_pmask):
    with (
        tc.tile_pool(name="dram", bufs=1, space="DRAM") as dram,
        tc.tile_pool(name="base", bufs=1) as base,
        tc.tile_pool(name="stats", bufs=4) as stats,
        tc.tile_pool(name="hp", bufs=1) as hp,
    ):
        eye = base.tile([128, 128], BF16)
        nc.sync.dma_start(eye[:], eye_in.ap())
        msel = base.tile([128, 2], F32)
        nc.sync.dma_start(msel[:], msel_in.ap())

        # o^T exchange staging: per pair [130, L] (rows 0:65 = head A
        # [64 dims + rowsum], 65:130 = head B), unnormalized; pair 3 is
        # split into two column halves so its AG pipelines earlier.
        stage = [dram.tile([130, L], BF16, tag=f"st{p}", name=f"st{p}")
                 for p in range(3)]
        rsinv_d = [dram.tile([4, RC], BF16, tag=f"rsd{p}", name=f"rsd{p}")
                   for p in range(4)]
        stage3 = [dram.tile([130, L // 4], BF16, tag=f"st3{i}", name=f"st3{i}")
                  for i in range(4)]
        agout = [dram.tile([260, L], BF16, tag=f"ago{p}", name=f"ago{p}")
                 for p in range(3)]
        agout3 = [dram.tile([260, L // 4], BF16, tag=f"ago3{i}",
                            name=f"ago3{i}") for i in range(4)]

        attn_stack = ExitStack()
        attn = attn_stack.enter_context(tc.tile_pool(name="attn", bufs=1))
        pm_t = [attn.tile([128, 1024], BF16, tag=f"pm{i}", name=f"pm{i}")
                for i in range(n_pmask)]
        # Q/K per pair, [128, L]: rows [0:64] head A (x1 dims 0:32,
        # x2 dims 32:64), rows [64:128] head B. Fully written by the
        # rope scatter -- no zero padding needed (64-row contraction).
        QT = [attn.tile([128, L], BF16, tag=f"qt{p}", name=f"qt{p}")
              for p in range(4)]
        KT = [attn.tile([128, L], BF16, tag=f"kt{p}", name=f"kt{p}")
              for p in range(4)]
        V_sb = [attn.tile([128, 16 * VCOL], BF16, tag=f"vs{p}", name=f"vs{p}")
                for p in range(4)]
        for p in range(4):
            # only the per-ktile ones-columns (64, 129 of each 130 block)
            # need initialization; V data columns are fully overwritten
            vv = V_sb[p][:].rearrange("a (kt c) -> a kt c", c=VCOL)
            nc.gpsimd.memset(vv[:, :, 64:65], 1.0)
            nc.gpsimd.memset(vv[:, :, 129:130], 1.0)

        # ---------------- phase 1+2: x load/rms/transpose software-
        # pipelined with per-pair QKV + RoPE + V transpose
        with (
            tc.tile_pool(name="xnt", bufs=1) as xnt_pool,
            tc.tile_pool(name="cst", bufs=1) as cst,
            tc.tile_pool(name="ps1t", bufs=2, space="PSUM") as ps1t,
            tc.tile_pool(name="ps1m", bufs=2, space="PSUM") as ps1m,
            tc.tile_pool(name="p1", bufs=3) as p1,
            tc.tile_pool(name="rp", bufs=2) as rp,
            tc.tile_pool(name="xp", bufs=5) as xp,
            tc.tile_pool(name="vtp", bufs=2) as vtp,
        ):
            cosT = cst.tile([128, L], BF16)
            sinT = cst.tile([128, L], BF16)
            w_t = [cst.tile([128, 4 * 384], BF16, tag=f"w{kc}", name=f"w{kc}")
                   for kc in range(8)]
            # constants on the sync queue (x rides the gpsimd queue, and
            # the ACT queue stays DMA-free so compute is never credit-blocked);
            # weights first (the first QKV group blocks on them)
            for kc in range(8):
                nc.sync.dma_start(w_t[kc][:],
                                  wqkv_in.ap()[128 * kc:128 * (kc + 1), :])
            nc.sync.dma_start(cosT[:], cos_in.ap())
            nc.sync.dma_start(sinT[:], sin_in.ap())
            for i in range(n_pmask):
                nc.sync.dma_start(pm_t[i][:], pm_in.ap()[i, :, :])

            # xnT_all[:, fc, l]: transposed normalized x (dim on
            # partitions), single tile so transpose copies can batch
            xnT = xnt_pool.tile([128, 8 * L], BF16, name="xnT")
            xnT_v = xnT[:].rearrange("a (fc l) -> a fc l", fc=8)

            xts = {}

            def emit_x_dma(rt):
                xt = xp.tile([128, DIM], F32, tag="x", name="xt")
                nc.gpsimd.dma_start(xt[:],
                                    xb_in.ap()[128 * rt:128 * (rt + 1), :])
                xts[rt] = xt

            def emit_tile(rt):
                xt = xts.pop(rt)
                rstd = _rms_rstd(nc, p1, stats, xt, RMS_EPS)
                xn = p1.tile([128, DIM], BF16, tag="xn", name="xn")
                nc.scalar.activation(xn[:], xt[:], AF.Copy, bias=0.0,
                                     scale=rstd[:])
                for half in range(2):
                    ps = ps1t.tile([128, 512], BF16, tag="tr", name="tr")
                    for j in range(4):
                        fc = 4 * half + j
                        nc.tensor.transpose(
                            ps[:, 128 * j:128 * (j + 1)],
                            xn[:, 128 * fc:128 * (fc + 1)], eye[:])
                    nc.vector.tensor_copy(
                        xnT_v[:, 4 * half:4 * half + 4,
                              128 * rt:128 * (rt + 1)],
                        ps[:].rearrange("a (j c) -> a j c", j=4))

            def emit_qkv_group(g):
                rb, p = g // 4, g % 4
                g0 = 512 * rb
                w0 = 384 * p
                psX = ps1m.tile([128, 1024], F32, tag="qk", name="psx")
                psX1 = psX[:, 0:512]
                psX2 = psX[:, 512:1024]
                psV = ps1m.tile([128, 512], F32, tag="v", name="psv")
                for kc in range(8):
                    st, sp = kc == 0, kc == 7
                    xs = xnT_v[:, kc, g0:g0 + 512]
                    nc.tensor.matmul(psX1, w_t[kc][:, w0:w0 + 128],
                                     xs, start=st, stop=sp)
                    nc.tensor.matmul(psX2, w_t[kc][:, w0 + 128:w0 + 256],
                                     xs, start=st, stop=sp)
                    nc.tensor.matmul(psV[:], w_t[kc][:, w0 + 256:w0 + 384],
                                     xs, start=st, stop=sp)
                # rope: psX1 rows = [qA_x1 qB_x1 kA_x1 kB_x1] (32 each),
                # psX2 = x2 counterparts. bf16 math after ACT casts.
                x1b = rp.tile([128, 512], BF16, tag="x1b", name="x1b")
                x2b = rp.tile([128, 512], BF16, tag="x2b", name="x2b")
                nc.scalar.activation(x1b[:], psX1, AF.Copy)
                nc.scalar.activation(x2b[:], psX2, AF.Copy)
                cs = cosT[:, g0:g0 + 512]
                sn = sinT[:, g0:g0 + 512]
                t1 = rp.tile([128, 512], BF16, tag="r1", name="r1")
                t2 = rp.tile([128, 512], BF16, tag="r2", name="r2")
                t3 = rp.tile([128, 512], BF16, tag="r3", name="r3")
                t4 = rp.tile([128, 512], BF16, tag="r4", name="r4")
                o1 = p1.tile([128, 512], BF16, tag="o1", name="o1")
                o2 = p1.tile([128, 512], BF16, tag="o2", name="o2")
                nc.vector.tensor_mul(t1[:], x1b[:], cs)
                nc.vector.tensor_mul(t2[:], x2b[:], sn)
                nc.vector.tensor_sub(o1[:], t1[:], t2[:])
                nc.vector.tensor_mul(t3[:], x1b[:], sn)
                nc.vector.tensor_mul(t4[:], x2b[:], cs)
                nc.vector.tensor_add(o2[:], t3[:], t4[:])
                for src_t, s0, dst, d0 in (
                    (o1, 0, QT[p], 0), (o2, 0, QT[p], 32),
                    (o1, 32, QT[p], 64), (o2, 32, QT[p], 96),
                    (o1, 64, KT[p], 0), (o2, 64, KT[p], 32),
                    (o1, 96, KT[p], 64), (o2, 96, KT[p], 96),
                ):
                    nc.sync.dma_start(dst[d0:d0 + 32, g0:g0 + 512],
                                      src_t[s0:s0 + 32, :])
                # V^T -> row-major V blocks, local 4 ktiles
                VT = vtp.tile([128, 512], BF16, tag="vt", name="vt")
                nc.scalar.activation(VT[:], psV[:], AF.Copy)
                psv_t = ps1t.tile([128, 512], BF16, tag="tr", name="vtr")
                for kk in range(4):
                    nc.tensor.transpose(
                        psv_t[:, 128 * kk:128 * (kk + 1)],
                        VT[:, 128 * kk:128 * (kk + 1)], eye[:])
                vv = V_sb[p][:].rearrange("a (kt c) -> a kt c", c=VCOL)
                pv = psv_t[:].rearrange("a (kk c) -> a kk c", c=128)
                nc.vector.tensor_copy(vv[:, 4 * rb:4 * rb + 4, 0:64],
                                      pv[:, :, 0:64])
                nc.vector.tensor_copy(vv[:, 4 * rb:4 * rb + 4, 65:129],
                                      pv[:, :, 64:128])

            for rt in range(4):
                emit_x_dma(rt)
            for i in range(16):
                if i + 4 < 16:
                    emit_x_dma(i + 4)
                emit_tile(i)
                if i >= 3:
                    emit_qkv_group(i - 3)
            for g in range(13, 16):
                emit_qkv_group(g)

        # ---------------- phase 3: attention per (pair, qblock, ktile),
        # o left unnormalized; rowsum rides along row 64 of each psO.
        # Per-pair AllToAll; selects for pair p are emitted after pair
        # p+1's attention so the in-order engine queues never stall on
        # collective latency.
        h_t = [hp.tile([128, DIM], F32, tag=f"h{rt}", name=f"h{rt}")
               for rt in range(8)]
        with (
            tc.tile_pool(name="p7", bufs=1) as p7,
            tc.tile_pool(name="p7w", bufs=2) as p7w,
        ):
            for rt in range(8):
                nc.gpsimd.dma_start(h_t[rt][:],
                                    x_in.ap()[128 * rt:128 * (rt + 1), :])
            ow_t = []
            for kc in range(8):
                w = p7w.tile([128, 1024], BF16, tag="ow", name="ow", bufs=8)
                nc.sync.dma_start(w[:],
                                  outw_in.ap()[128 * kc:128 * (kc + 1), :])
                ow_t.append(w)
            oT = [None] * 8

            def emit_select(p):
                rs4 = p7w.tile([4, L], BF16, tag="rs4", name="rs4", bufs=1)
                if p < 3:
                    ag = agout[p]
                    for j, row in enumerate((64, 129, 194, 259)):
                        nc.sync.dma_start(rs4[j:j + 1, :], ag[row:row + 1, :])
                else:
                    for ih, ag in enumerate(agout3):
                        c0 = 512 * ih
                        for j, row in enumerate((64, 129, 194, 259)):
                            nc.sync.dma_start(rs4[j:j + 1, c0:c0 + 512],
                                              ag[row:row + 1, :])
                # data-driven column select of the rowsums, then 1/x
                rs0 = p7w.tile([4, RC], F32, tag="rs0", name="rs0", bufs=1)
                nc.vector.tensor_scalar(rs0[:], rs4[:, 0:RC],
                                        msel[0:4, 0:1], None, ALU.mult)
                nc.vector.scalar_tensor_tensor(rs0[:], rs4[:, RC:2 * RC],
                                               msel[0:4, 1:2], rs0[:],
                                               ALU.mult, ALU.add)
                rsinv = p7w.tile([4, RC], F32, tag="rsinv", name="rsinv",
                                 bufs=1)
                nc.vector.reciprocal_approx_fast(rsinv[:], rs0[:])
                rsinvb = p7w.tile([4, RC], BF16, tag="rsinvb", name="rsinvb",
                                  bufs=1)
                nc.vector.tensor_copy(rsinvb[:], rsinv[:])
                nc.sync.dma_start(rsinv_d[p][:], rsinvb[:])
                for s in range(2):
                    kc = 4 * s + p
                    tf = p7w.tile([128, L], BF16, tag="tf", name="tf",
                                  bufs=2)
                    if p < 3:
                        ag = agout[p]
                        nc.sync.dma_start(tf[0:64, :],
                                          ag[130 * s:130 * s + 64, :])
                        nc.sync.dma_start(tf[64:128, :],
                                          ag[130 * s + 65:130 * s + 129, :])
                    else:
                        for ih, ag in enumerate(agout3):
                            c0 = 512 * ih
                            nc.sync.dma_start(tf[0:64, c0:c0 + 512],
                                              ag[130 * s:130 * s + 64, :])
                            nc.sync.dma_start(tf[64:128, c0:c0 + 512],
                                              ag[130 * s + 65:130 * s + 129, :])
                    # 1/rowsum broadcast across each head's 64 partitions
                    # via stride-0 DMA source
                    Mb = p7w.tile([128, RC], BF16, tag="mb", name="mb",
                                  bufs=2)
                    nc.sync.dma_start(
                        Mb[0:64, :],
                        rsinv_d[p][2 * s:2 * s + 1, :].to_broadcast((64, RC)))
                    nc.sync.dma_start(
                        Mb[64:128, :],
                        rsinv_d[p][2 * s + 1:2 * s + 2, :]
                        .to_broadcast((64, RC)))
                    # column-select fused with normalization
                    ta = p7w.tile([128, RC], BF16, tag="ta", name="ta",
                                  bufs=1)
                    tb = p7w.tile([128, RC], BF16, tag="tb2", name="tb2",
                                  bufs=1)
                    ot = p7.tile([128, RC], BF16, tag=f"ot{kc}",
                                 name=f"oT{kc}")
                    nc.vector.scalar_tensor_tensor(
                        ta[:], tf[:, 0:RC], msel[:, 0:1], Mb[:],
                        ALU.mult, ALU.mult)
                    nc.vector.scalar_tensor_tensor(
                        tb[:], tf[:, RC:2 * RC], msel[:, 1:2], Mb[:],
                        ALU.mult, ALU.mult)
                    nc.vector.tensor_add(ot[:], ta[:], tb[:])
                    oT[kc] = ot

            with (
                tc.tile_pool(name="ps5s", bufs=2, space="PSUM") as ps5s,
                tc.tile_pool(name="ps5o", bufs=2, space="PSUM") as ps5o,
                tc.tile_pool(name="p5", bufs=3) as p5,
            ):
                LOOK = 4
                rgs = [[2 * i, 2 * i + 1] for i in range(4)]
                for p in range(4):
                    for qb in range(4):
                        act = [(kt, actions[(kt, qb)]) for kt in range(16)
                               if actions[(kt, qb)] != "skip"]
                        n = len(act)
                        psOa = ps5o.tile([65, 512], F32, tag="oa", name="psoa",
                                         bufs=2)
                        psOb = ps5o.tile([65, 512], F32, tag="ob", name="psob",
                                         bufs=2)
                        pts = []
                        for i in range(n + LOOK):
                            if i < n:
                                kt, a = act[i]
                                psS = ps5s.tile([128, 1024], F32, tag="s",
                                                name="pss", bufs=2)
                                nc.tensor.matmul(
                                    psS[:, 0:512],
                                    KT[p][0:64, 128 * kt:128 * (kt + 1)],
                                    QT[p][0:64, 512 * qb:512 * (qb + 1)],
                                    start=True, stop=True)
                                nc.tensor.matmul(
                                    psS[:, 512:1024],
                                    KT[p][64:128, 128 * kt:128 * (kt + 1)],
                                    QT[p][64:128, 512 * qb:512 * (qb + 1)],
                                    start=True, stop=True)
                                pt = p5.tile([128, 1024], BF16, tag="pt",
                                             name="pt", bufs=8)
                                nc.scalar.activation(pt[:], psS[:], AF.Exp,
                                                     bias=0.0, scale=0.125)
                                if a != "noop":
                                    nc.vector.tensor_mul(pt[:], pt[:],
                                                         pm_t[a[1]][:])
                                pts.append((kt, pt))
                            j = i - LOOK
                            if 0 <= j < n:
                                kt, pt = pts[j]
                                nc.tensor.matmul(
                                    psOa[:],
                                    V_sb[p][:, VCOL * kt:VCOL * kt + 65],
                                    pt[:, 0:512],
                                    start=(j == 0), stop=(j == n - 1))
                                nc.tensor.matmul(
                                    psOb[:],
                                    V_sb[p][:, VCOL * kt + 65:VCOL * kt + 130],
                                    pt[:, 512:1024],
                                    start=(j == 0), stop=(j == n - 1))
                        oA = p5.tile([65, 512], BF16, tag="oA", name="oA",
                                     bufs=4)
                        oB = p5.tile([65, 512], BF16, tag="oB", name="oB",
                                     bufs=4)
                        nc.vector.tensor_copy(oA[:], psOa[:])
                        nc.vector.tensor_copy(oB[:], psOb[:])
                        if p < 3:
                            nc.gpsimd.dma_start(
                                stage[p][0:65, 512 * qb:512 * (qb + 1)],
                                oA[:])
                            nc.gpsimd.dma_start(
                                stage[p][65:130, 512 * qb:512 * (qb + 1)],
                                oB[:])
                        else:
                            st3 = stage3[qb]
                            nc.gpsimd.dma_start(st3[0:65, :], oA[:])
                            nc.gpsimd.dma_start(st3[65:130, :], oB[:])
                            nc.gpsimd.collective_compute(
                                "AllGather", ALU.bypass, replica_groups=rgs,
                                ins=[stage3[qb][:].opt()],
                                outs=[agout3[qb][:].opt()])
                    if p < 3:
                        nc.gpsimd.collective_compute(
                            "AllGather", ALU.bypass, replica_groups=rgs,
                            ins=[stage[p][:].opt()],
                            outs=[agout[p][:].opt()])
                    if p >= 1:
                        emit_select(p - 1)
                emit_select(3)

            # ---------------- phase 4: out-proj + residual
            with tc.tile_pool(name="ps7", bufs=1, space="PSUM") as ps7:
                kc_order = [0, 4, 1, 5, 2, 6, 3, 7]
                for nb in range(2):
                    pss = [ps7.tile([128, 512], F32, tag=f"mm{rt % 4}",
                                    name="psmm", bufs=2) for rt in range(8)]
                    for ki, kc in enumerate(kc_order):
                        w = ow_t[kc][:, 512 * nb:512 * (nb + 1)]
                        for rt in range(8):
                            nc.tensor.matmul(pss[rt][:],
                                             oT[kc][:, 128 * rt:128 * (rt + 1)],
                                             w, start=(ki == 0), stop=(ki == 7))
                    for rt in range(8):
                        nc.vector.tensor_add(
                            h_t[rt][:, 512 * nb:512 * (nb + 1)], pss[rt][:],
                            h_t[rt][:, 512 * nb:512 * (nb + 1)])

        # free Q/K/V/pmask before the FFN working set opens
        attn_stack.close()

        # ------------ phase 5: FFN (row-local), two halves of 512 rows
        with (
            tc.tile_pool(name="ps8", bufs=1, space="PSUM") as ps8,
            tc.tile_pool(name="ps8t", bufs=3, space="PSUM") as ps8t,
            tc.tile_pool(name="p8", bufs=1) as p8,
            tc.tile_pool(name="p8w", bufs=2) as p8w,
            tc.tile_pool(name="p8s", bufs=3) as p8s,
        ):
            for half in range(2):
                # rms-norm h -> fn (bf16) -> transpose -> fnT_all
                fnT = p8.tile([128, 8 * 512], BF16, tag="fnT", name="fnT")
                fnT_v = fnT[:].rearrange("a (fc l) -> a fc l", fc=8)
                for rt2 in range(4):
                    rt = 4 * half + rt2
                    rstd = _rms_rstd(nc, p8s, stats, h_t[rt], RMS_EPS)
                    fn = p8s.tile([128, DIM], BF16, tag="fn", name="fn")
                    nc.scalar.activation(fn[:], h_t[rt][:], AF.Copy,
                                         bias=0.0, scale=rstd[:])
                    for hf in range(2):
                        ps = ps8t.tile([128, 512], BF16, tag="tr", name="tr")
                        for j in range(4):
                            fc = 4 * hf + j
                            nc.tensor.transpose(
                                ps[:, 128 * j:128 * (j + 1)],
                                fn[:, 128 * fc:128 * (fc + 1)], eye[:])
                        nc.vector.tensor_copy(
                            fnT_v[:, 4 * hf:4 * hf + 4,
                                  128 * rt2:128 * (rt2 + 1)],
                            ps[:].rearrange("a (j c) -> a j c", j=4))
                # lin1 + GELU -> g; bn_stats inline after each gelu
                g_t = [[p8.tile([128, 512], BF16, tag=f"g{rt2}_{hb}",
                                name=f"g{rt2}_{hb}")
                        for hb in range(8)] for rt2 in range(4)]
                st_t = [p8.tile([128, 8, 6], F32, tag=f"lnst{rt2}",
                                name=f"lnst{rt2}") for rt2 in range(4)]
                for hb in range(8):
                    pss = [ps8.tile([128, 512], F32, tag=f"mm{rt2}",
                                    name="psmm", bufs=1)
                           for rt2 in range(4)]
                    wa = p8w.tile([128, 4096], BF16, tag="l1w",
                                  name="l1w", bufs=2)
                    nc.gpsimd.dma_start(
                        wa[:].rearrange("a (fc c) -> a fc c", fc=8),
                        l1w_in.ap()[:, 512 * hb:512 * (hb + 1)]
                        .rearrange("(fc a) c -> a fc c", a=128))
                    for fc in range(8):
                        for rt2 in range(4):
                            nc.tensor.matmul(
                                pss[rt2][:],
                                fnT_v[:, fc, 128 * rt2:128 * (rt2 + 1)],
                                wa[:, 512 * fc:512 * (fc + 1)],
                                start=(fc == 0), stop=(fc == 7))
                    for rt2 in range(4):
                        nc.scalar.activation(g_t[rt2][hb][:], pss[rt2][:],
                                             AF.Gelu)
                        nc.vector.bn_stats(st_t[rt2][:, hb, :],
                                           g_t[rt2][hb][:])
                # LayerNorm stats over hid (4096) per row
                ab = []
                for rt2 in range(4):
                    mv = stats.tile([128, 2], F32, tag="lnmv", name="lnmv")
                    nc.vector.bn_aggr(mv[:], st_t[rt2][:])
                    std = stats.tile([128, 1], F32, tag="lnsd", name="lnsd")
                    nc.scalar.activation(std[:], mv[:, 1:2], AF.Sqrt,
                                         bias=LN_EPS, scale=1.0)
                    rstd = stats.tile([128, 1], F32, tag="lnrs", name="lnrs")
                    nc.vector.reciprocal(rstd[:], std[:])
                    nmr = stats.tile([128, 1], F32, tag="lnnm", name="lnnm")
                    nc.vector.tensor_scalar(nmr[:], rstd[:], mv[:, 0:1],
                                            -1.0, ALU.mult, ALU.mult)
                    ab.append((rstd, nmr))
                # normalize + transpose -> gnT_all, with lin2 nb=0 matmuls
                # interleaved per hb so the PE never waits on the LN chain
                gnT = p8.tile([128, 32 * 512], BF16, tag="gnT", name="gnT")
                gnT_v = gnT[:].rearrange("a (hc l) -> a hc l", hc=32)
                pss0 = [ps8.tile([128, 512], F32, tag=f"mm{rt2}",
                                 name="psmm", bufs=1)
                        for rt2 in range(4)]
                for hb in range(8):
                    for rt2 in range(4):
                        rstd, nmr = ab[rt2]
                        gn = p8s.tile([128, 512], BF16, tag="gn", name="gn")
                        nc.scalar.activation(gn[:], g_t[rt2][hb][:],
                                             AF.Identity, bias=nmr[:],
                                             scale=rstd[:])
                        ps = ps8t.tile([128, 512], BF16, tag="tr", name="tr")
                        for j in range(4):
                            nc.tensor.transpose(
                                ps[:, 128 * j:128 * (j + 1)],
                                gn[:, 128 * j:128 * (j + 1)], eye[:])
                        nc.vector.tensor_copy(
                            gnT_v[:, 4 * hb:4 * hb + 4,
                                  128 * rt2:128 * (rt2 + 1)],
                            ps[:].rearrange("a (j c) -> a j c", j=4))
                    wa = p8w.tile([128, 2048], BF16, tag="l2w",
                                  name="l2w", bufs=3)
                    nc.gpsimd.dma_start(
                        wa[:].rearrange("a (j c) -> a j c", j=4),
                        l2w_in.ap()[512 * hb:512 * (hb + 1), 0:512]
                        .rearrange("(j a) c -> a j c", a=128))
                    for j in range(4):
                        hc = 4 * hb + j
                        for rt2 in range(4):
                            nc.tensor.matmul(
                                pss0[rt2][:],
                                gnT_v[:, hc, 128 * rt2:128 * (rt2 + 1)],
                                wa[:, 512 * j:512 * (j + 1)],
                                start=(hc == 0), stop=(hc == 31))
                for rt2 in range(4):
                    rt = 4 * half + rt2
                    yt = p8s.tile([128, 512], F32, tag="yt", name="yt")
                    nc.vector.tensor_add(yt[:], pss0[rt2][:],
                                         h_t[rt][:, 0:512])
                    nc.sync.dma_start(
                        y_out.ap()[128 * rt:128 * (rt + 1), 0:512], yt[:])
                # lin2 nb=1: dense matmul pass
                pss1 = [ps8.tile([128, 512], F32, tag=f"mm{rt2}",
                                 name="psmm", bufs=1)
                        for rt2 in range(4)]
                for hb in range(8):
                    wa = p8w.tile([128, 2048], BF16, tag="l2w",
                                  name="l2w", bufs=3)
                    nc.gpsimd.dma_start(
                        wa[:].rearrange("a (j c) -> a j c", j=4),
                        l2w_in.ap()[512 * hb:512 * (hb + 1), 512:1024]
                        .rearrange("(j a) c -> a j c", a=128))
                    for j in range(4):
                        hc = 4 * hb + j
                        for rt2 in range(4):
                            nc.tensor.matmul(
                                pss1[rt2][:],
                                gnT_v[:, hc, 128 * rt2:128 * (rt2 + 1)],
                                wa[:, 512 * j:512 * (j + 1)],
                                start=(hc == 0), stop=(hc == 31))
                for rt2 in range(4):
                    rt = 4 * half + rt2
                    yt = p8s.tile([128, 512], F32, tag="yt", name="yt")
                    nc.vector.tensor_add(yt[:], pss1[rt2][:],
                                         h_t[rt][:, 512:1024])
                    nc.sync.dma_start(
                        y_out.ap()[128 * rt:128 * (rt + 1), 512:1024], yt[:])


# ----------------------------------------------------------------------------
# entry point
# ----------------------------------------------------------------------------

def kernel(x, mask, attn_scale, wqkv_w, wqkv_b, out_w, out_b,
           ffn_scale, lin1_w, lin1_b, ln_g, ln_b, lin2_w, lin2_b):
    x = np.asarray(x, np.float32)
    mask = np.asarray(mask, np.float32)

    lin2_b_eff = (np.asarray(lin2_b, np.float32)
                  + np.asarray(ln_b, np.float32) @ np.asarray(lin2_w, np.float32))
    if np.any(wqkv_b) or np.any(out_b) or np.any(lin1_b) or np.any(lin2_b_eff):
        return _numpy_fallback(x, mask, attn_scale, wqkv_w, wqkv_b, out_w, out_b,
                               ffn_scale, lin1_w, lin1_b, ln_g, ln_b, lin2_w,
                               lin2_b)

    actions, pmask_np = _classify_mask(mask)
    for qb in range(4):
        if all(actions[(kt, qb)] == "skip" for kt in range(16)):
            return _numpy_fallback(x, mask, attn_scale, wqkv_w, wqkv_b, out_w,
                                   out_b, ffn_scale, lin1_w, lin1_b, ln_g, ln_b,
                                   lin2_w, lin2_b)

    mask_sig = tuple(sorted((k, str(v)) for k, v in actions.items()))
    key = (mask_sig, pmask_np.shape[0])
    if key not in _PROGRAM_CACHE:
        _PROGRAM_CACHE[key] = _build_program(actions, pmask_np.shape[0])
    nc = _PROGRAM_CACHE[key]

    asc = np.asarray(attn_scale, np.float32)
    wqkv_eff = asc[:, None] * np.asarray(wqkv_w, np.float32)
    wq, wk, wv = (wqkv_eff[:, :DIM], wqkv_eff[:, DIM:2 * DIM],
                  wqkv_eff[:, 2 * DIM:])
    out_w_bf = _bf16(out_w)
    l1_bf = _bf16(np.asarray(ffn_scale, np.float32)[:, None]
                  * np.asarray(lin1_w, np.float32))
    l2_bf = _bf16(np.asarray(lin2_w, np.float32)
                  * np.asarray(ln_g, np.float32)[:, None])
    cosT, sinT = _rope_tables()
    eye = np.eye(128, dtype=ml_dtypes.bfloat16)

    x2 = np.ascontiguousarray(x.reshape(R, DIM))
    in_maps = []
    for c in range(NCORES):
        b, g = c // 2, c % 2
        sls = []
        for p in range(4):
            hA, hB = 8 * g + 2 * p, 8 * g + 2 * p + 1
            qA, qB = wq[:, 64 * hA:64 * hA + 64], wq[:, 64 * hB:64 * hB + 64]
            kA, kB = wk[:, 64 * hA:64 * hA + 64], wk[:, 64 * hB:64 * hB + 64]
            vA, vB = wv[:, 64 * hA:64 * hA + 64], wv[:, 64 * hB:64 * hB + 64]
            sls.append(np.concatenate(
                [qA[:, :32], qB[:, :32], kA[:, :32], kB[:, :32],
                 qA[:, 32:], qB[:, 32:], kA[:, 32:], kB[:, 32:],
                 vA, vB], axis=1))
        sl = np.concatenate(sls, axis=1)          # [1024, 1536]
        msel_np = np.zeros((128, 2), np.float32)
        msel_np[:, g] = 1.0
        in_maps.append(dict(
            x_batch=np.ascontiguousarray(x2[L * b:L * (b + 1)]),
            x_own=np.ascontiguousarray(x2[RC * c:RC * (c + 1)]),
            wqkv_sl=_bf16(sl),
            out_w=out_w_bf,
            lin1_w=l1_bf,
            lin2_w=l2_bf,
            cosT=cosT,
            sinT=sinT,
            pmask=pmask_np,
            eye=eye,
            msel=msel_np,
        ))

    global _LAST_IN_MAPS
    _LAST_IN_MAPS = in_maps
    res = run_bass_kernel_spmd(nc, in_maps, core_ids=list(range(NCORES)))
    y = np.concatenate([res.results[c]["y_own"] for c in range(NCORES)], axis=0)
    return y.reshape(B, L, DIM).astype(np.float32)
